# revision 5
# baseline (speedup 1.0000x reference)
"""AttentionBlock kernel for Trainium2 (Bass/Tile), data-parallel over batch.

Shapes (hardcoded): x (8, 256, 32, 32); Wp (256, 768); bp (768,);
Wo (256, 256); bo (256,). Output (8, 256, 32, 32) fp32.

Each of the 8 NeuronCores processes one batch element. Per core everything is
kept in the "transposed" domain (channels on partitions), which matches both
the input layout x[b] = xs^T = [C, N] and the required output layout out^T:

  q^T, k^T [256, 1024] (head-grouped rows: row h*64+d), v [1024, 256] natural
  S^T_h = (k_h^T).T @ q_h^T  -> [1024(j), 1024(i)]   (K=64 matmuls)
  E^T = exp(S^T / 8)  (ScalarE, straight out of PSUM; softmax max-sub skipped:
                       inputs are ~N(0,1) so scores are far from fp32 overflow)
  [U^T; Z] = accumulated with lhsT = [v_h | 1] (M=65): U rows 0-63, Z row 64
  res^T_h = U^T_h * partition_broadcast(1/Z)
  out^T = Wo^T res^T + bo + xs^T

Matmul operands are float32r (TF32-like, 1 cycle/row vs 4 for exact fp32);
the BIR verifier requires them to be produced by a rounding compute op, so
DMA-loaded tensors get a DVE rounding copy first.
"""

import numpy as np

NUM_HEADS = 4
HEAD_DIM = 64
C = 256
N = 1024
B = 8
N_CORES = 8

# matmul input dtype: "f32r" (1 cycle/row, TF32-like precision) or "f32"
# (exact fp32, 4 cycles/row).
MM_MODE = "f32r"

_CACHE = {}


def _emit_body(nc, tc, aps, pools, mm_mode, rep):
    import concourse.bass as bass
    import concourse.mybir as mybir

    f32 = mybir.dt.float32
    mmdt = mybir.dt.float32r if mm_mode == "f32r" else f32
    Exp = mybir.ActivationFunctionType.Exp
    add = mybir.AluOpType.add
    consts, etp, normp, ps_qkv, ps_s, ps_u = pools
    x_d, wq_d, wk_d, wv_d, wo_d, bq_d, bk_d, bv_d, bo_d, out_d = aps
    r = f"_{rep}"

    # ---- load inputs -----------------------------------------------------
    x_sb = consts.tile([128, 2, N], f32, tag="x_sb", name="x_sb" + r)
    nc.sync.dma_start(out=x_sb, in_=x_d.rearrange("(ko ki) n -> ki ko n", ki=128))

    w_sbs = {}
    for name, w_d in (("wq", wq_d), ("wk", wk_d), ("wv", wv_d), ("wo", wo_d)):
        w_sb = consts.tile([128, 2, C], f32, tag=name, name=name + r)
        nc.sync.dma_start(out=w_sb, in_=w_d.rearrange("(ko ki) f -> ki ko f", ki=128))
        w_sbs[name] = w_sb

    # rounded copies for matmul consumption (f32r mode)
    if mm_mode == "f32r":
        x_r = consts.tile([128, 2, N], mmdt, tag="x_r", name="x_r" + r)
        nc.vector.tensor_copy(x_r, x_sb)
        w_rs = {}
        for name in ("wq", "wk", "wv", "wo"):
            w_r = consts.tile([128, 2, C], mmdt, tag=name + "r", name=name + "r" + r)
            nc.vector.tensor_copy(w_r, w_sbs[name])
            w_rs[name] = w_r
    else:
        x_r = x_sb
        w_rs = w_sbs
    wq_r, wk_r, wv_r, wo_r = (w_rs[k] for k in ("wq", "wk", "wv", "wo"))

    b_sbs = {}
    for name, b_d in (("bq", bq_d), ("bk", bk_d), ("bo", bo_d)):
        b_sb = consts.tile([128, 2], f32, tag=name, name=name + r)
        nc.sync.dma_start(out=b_sb, in_=b_d.rearrange("(fo fi) -> fi fo", fi=128))
        b_sbs[name] = b_sb
    bq_sb, bk_sb, bo_sb = (b_sbs[k] for k in ("bq", "bk", "bo"))

    # bv broadcast across partitions (used along the free axis of v)
    bv_bc = consts.tile([128, C], f32, tag="bv_bc", name="bv_bc" + r)
    nc.sync.dma_start(
        out=bv_bc,
        in_=bass.AP(tensor=bv_d.tensor, offset=bv_d.offset, ap=[[0, 128], [1, C]]),
    )

    # ---- QKV projections -------------------------------------------------
    qT_sb = consts.tile([128, 2, N], mmdt, tag="qT", name="qT" + r)
    kT_sb = consts.tile([128, 2, N], mmdt, tag="kT", name="kT" + r)
    # v natural [n, hd] + ones column per head: [ni, nt, h, 64+1]
    v_sb = consts.tile([128, 8, NUM_HEADS, HEAD_DIM + 1], mmdt, tag="v", name="v" + r)
    ones_c = consts.tile([128, 1], f32, tag="ones", name="ones" + r)
    nc.vector.memset(ones_c, 1.0)
    nc.vector.tensor_copy(
        out=v_sb[:, :, :, HEAD_DIM : HEAD_DIM + 1],
        in_=ones_c.to_broadcast((128, 8, NUM_HEADS, 1)),
    )

    for w_r, b_sb, dst in ((wq_r, bq_sb, qT_sb), (wk_r, bk_sb, kT_sb)):
        for ft in range(2):
            for ic in range(2):
                ps = ps_qkv.tile(
                    [128, 512], f32, tag="pqkv", name=f"pq_{dst.name}_{ft}_{ic}{r}"
                )
                for ko in range(2):
                    nc.tensor.matmul(
                        ps,
                        lhsT=w_r[:, ko, ft * 128 : (ft + 1) * 128],
                        rhs=x_r[:, ko, ic * 512 : (ic + 1) * 512],
                        start=(ko == 0),
                        stop=(ko == 1),
                    )
                nc.vector.tensor_scalar_add(
                    dst[:, ft, ic * 512 : (ic + 1) * 512], ps, b_sb[:, ft : ft + 1]
                )

    for nt in range(8):
        psv = ps_qkv.tile([128, 512], f32, tag="pqkv", name=f"pv_{nt}{r}")
        for ko in range(2):
            nc.tensor.matmul(
                psv[:, 0:C],
                lhsT=x_r[:, ko, nt * 128 : (nt + 1) * 128],
                rhs=wv_r[:, ko, :],
                start=(ko == 0),
                stop=(ko == 1),
            )
        nc.vector.tensor_add(
            out=v_sb[:, nt, :, 0:HEAD_DIM],
            in0=psv[:, 0:C].rearrange("p (h d) -> p h d", h=NUM_HEADS),
            in1=bv_bc.rearrange("p (h d) -> p h d", h=NUM_HEADS),
        )

    # ---- attention, head pair t = (2t, 2t+1) -----------------------------
    resT_sb = consts.tile([128, 2, N], mmdt, tag="resT", name="resT" + r)

    for t in range(2):
        heads = (2 * t, 2 * t + 1)
        eTs = [
            etp.tile([128, 8, N], mmdt, tag=f"eT{h % 2}", name=f"eT_{h}{r}")
            for h in heads
        ]

        # S^T + exp; pair interleaved so PE overlaps row groups 0-63/64-127
        for jt in range(8):
            pss = [
                ps_s.tile([128, N], f32, tag="ps_s", name=f"pss_{t}_{jt}_{i2}{r}")
                for i2 in range(2)
            ]
            for ic in range(2):
                for i, h in enumerate(heads):
                    b0 = 64 * (h % 2)
                    nc.tensor.matmul(
                        pss[i][:, ic * 512 : (ic + 1) * 512],
                        lhsT=kT_sb[b0 : b0 + 64, t, jt * 128 : (jt + 1) * 128],
                        rhs=qT_sb[b0 : b0 + 64, t, ic * 512 : (ic + 1) * 512],
                        start=True,
                        stop=True,
                    )
            for i, h in enumerate(heads):
                nc.scalar.activation(
                    out=eTs[i][:, jt, :], in_=pss[i], func=Exp, scale=0.125
                )

        # P @ [v | 1]: psum rows 0-63 = U^T_h, row 64 = Z
        for ic in range(2):
            psus = [
                ps_u.tile([128, 512], f32, tag="ps_u", name=f"psu_{t}_{ic}_{i2}{r}")
                for i2 in range(2)
            ]
            for jt in range(8):
                for i, h in enumerate(heads):
                    nc.tensor.matmul(
                        psus[i][0:65, :],
                        lhsT=v_sb[:, jt, h, :],
                        rhs=eTs[i][:, jt, ic * 512 : (ic + 1) * 512],
                        start=(jt == 0),
                        stop=(jt == 7),
                    )
            for i, h in enumerate(heads):
                b0 = 64 * (h % 2)
                # 1/Z into partition 0 (cross-base OK: single PSUM input), then
                # gpsimd broadcast from partition 0 (base-64 sources are broken
                # on HW).
                rz = normp.tile([128, 512], f32, tag="rz", name=f"rz_{t}_{ic}_{i}{r}")
                nc.vector.reciprocal(rz[0:1, :], psus[i][64:65, :])
                zb = normp.tile([128, 512], f32, tag="zb", name=f"zb_{t}_{ic}_{i}{r}")
                nc.gpsimd.partition_broadcast(zb, rz[0:1, :])
                nc.vector.tensor_mul(
                    resT_sb[b0 : b0 + 64, t, ic * 512 : (ic + 1) * 512],
                    psus[i][0:64, :],
                    zb[b0 : b0 + 64, :],
                )

    # ---- output projection + bias + residual -----------------------------
    out_sb = consts.tile([128, 2, N], f32, tag="out_sb", name="out_sb" + r)
    for ct in range(2):
        pso = ps_s.tile([128, N], f32, tag="ps_s", name=f"pso_{ct}{r}")
        for ic in range(2):
            for ko in range(2):
                nc.tensor.matmul(
                    pso[:, ic * 512 : (ic + 1) * 512],
                    lhsT=wo_r[:, ko, ct * 128 : (ct + 1) * 128],
                    rhs=resT_sb[:, ko, ic * 512 : (ic + 1) * 512],
                    start=(ko == 0),
                    stop=(ko == 1),
                )
        nc.vector.scalar_tensor_tensor(
            out=out_sb[:, ct, :],
            in0=pso,
            scalar=bo_sb[:, ct : ct + 1],
            in1=x_sb[:, ct, :],
            op0=add,
            op1=add,
        )

    nc.sync.dma_start(out=out_d.rearrange("(co ci) n -> ci co n", ci=128), in_=out_sb)


def _build_nc(mm_mode=MM_MODE, reps=1):
    import concourse.mybir as mybir
    import concourse.tile as tile
    from concourse import bacc
    from concourse._compat import axon_active

    f32 = mybir.dt.float32

    nc = bacc.Bacc(
        "TRN2",
        target_bir_lowering=False,
        debug=not axon_active(),
        num_devices=N_CORES,
    )

    aps = tuple(
        nc.dram_tensor(name, shape, f32, kind=kind).ap()
        for name, shape, kind in (
            ("x", [C, N], "ExternalInput"),
            ("wq", [C, C], "ExternalInput"),
            ("wk", [C, C], "ExternalInput"),
            ("wv", [C, C], "ExternalInput"),
            ("wo", [C, C], "ExternalInput"),
            ("bq", [C], "ExternalInput"),
            ("bk", [C], "ExternalInput"),
            ("bv", [C], "ExternalInput"),
            ("bo", [C], "ExternalInput"),
            ("out", [C, N], "ExternalOutput"),
        )
    )

    with tile.TileContext(nc) as tc:
        with (
            tc.tile_pool(name="consts", bufs=1) as consts,
            tc.tile_pool(name="et", bufs=1) as etp,
            tc.tile_pool(name="norm", bufs=2) as normp,
            tc.tile_pool(name="ps_qkv", bufs=2, space="PSUM") as ps_qkv,
            tc.tile_pool(name="ps_s", bufs=2, space="PSUM") as ps_s,
            tc.tile_pool(name="ps_u", bufs=2, space="PSUM") as ps_u,
        ):
            pools = (consts, etp, normp, ps_qkv, ps_s, ps_u)
            for rep in range(reps):
                _emit_body(nc, tc, aps, pools, mm_mode, rep)

    nc.compile()
    return nc


def get_nc(mm_mode=MM_MODE, reps=1):
    key = (mm_mode, reps)
    if key not in _CACHE:
        _CACHE[key] = _build_nc(mm_mode, reps)
    return _CACHE[key]


def make_in_maps(x, Wp, bp, Wo, bo):
    x = np.ascontiguousarray(x, dtype=np.float32)
    Wp3 = np.asarray(Wp, dtype=np.float32).reshape(C, NUM_HEADS, 3, HEAD_DIM)
    bp3 = np.asarray(bp, dtype=np.float32).reshape(NUM_HEADS, 3, HEAD_DIM)
    shared = {
        "wq": np.ascontiguousarray(Wp3[:, :, 0, :].reshape(C, C)),
        "wk": np.ascontiguousarray(Wp3[:, :, 1, :].reshape(C, C)),
        "wv": np.ascontiguousarray(Wp3[:, :, 2, :].reshape(C, C)),
        "wo": np.ascontiguousarray(Wo, dtype=np.float32),
        "bq": np.ascontiguousarray(bp3[:, 0, :].reshape(C)),
        "bk": np.ascontiguousarray(bp3[:, 1, :].reshape(C)),
        "bv": np.ascontiguousarray(bp3[:, 2, :].reshape(C)),
        "bo": np.ascontiguousarray(bo, dtype=np.float32),
    }
    return [
        {"x": np.ascontiguousarray(x[b].reshape(C, N)), **shared} for b in range(B)
    ]


def kernel(x, Wp, bp, Wo, bo):
    from concourse import bass_utils

    nc = get_nc()
    in_maps = make_in_maps(x, Wp, bp, Wo, bo)
    res = bass_utils.run_bass_kernel_spmd(nc, in_maps, core_ids=list(range(N_CORES)))
    out = np.stack([res.results[b]["out"] for b in range(B)])
    return out.reshape(B, C, 32, 32).astype(np.float32)
